# revision 30
# baseline (speedup 1.0000x reference)
"""Trainium2 Bass kernel for MoE-LoRA routing layer (nn_MoELoRA_47871705481666).

Computation (per token):
  z = x @ A_w.T                  (T, 512)   rank-8 slices for 64 experts
  logits = x @ (Wr1 @ Wr2)       (T, 64)    low-rank router
  probs = softmax(logits); gate = top16-masked probs (dense, not renormalized)
  out = (z * gate_expanded) @ B_w.T * 2.0
  aux = E * sum(f * p),  f = top16 count freq, p = mean probs

Distribution: data-parallel over tokens, 8 cores x 2048 tokens.  Weights
replicated (host pre-transposes/pre-combines them).  Per-core stat partial
sums (col-sums of probs and top16 indicators) are reduced on the host.

Precision: router matmul in fp32 (top-k selection fidelity), heavy matmuls
(z, out) in float32r (11-bit mantissa, full PE rate at N>=512).  x^T is
evicted from the transpose PSUM twice: an f32r copy for z (DVE, on the
critical path) and an fp32 copy for the router (ACT).
"""

import numpy as np

# Problem shapes (hardcoded per contract)
B, S, D = 4, 4096, 2048
E, R, TOPK, RDIM = 64, 8, 16, 16
ER = E * R            # 512
SCALING = 32.0 / TOPK  # 2.0
NCORES = 8
T = B * S             # 16384
TC = T // NCORES      # 2048 tokens per core
P = 128               # partitions

_BUILT = {}


def _build_nc(tc_tokens=TC):
    """Build the per-core Bass module."""
    from contextlib import ExitStack

    import concourse.bacc as bacc
    import concourse.tile as tile
    import concourse.mybir as mybir
    from concourse import masks

    f32 = mybir.dt.float32
    f32r = mybir.dt.float32r
    f16 = mybir.dt.float16
    FX = mybir.ActivationFunctionType
    ALU = mybir.AluOpType
    AX = mybir.AxisListType

    TT = 512                      # tokens per macro tile
    MT = tc_tokens // TT          # macro tiles
    NSUB = TT // P                # 4 subtiles of 128 tokens
    DCH = D // P                  # 16 d-chunks
    KCH = ER // P                 # 4 k-chunks
    OCH = D // 512                # 4 output chunks of 512
    AG = 4                        # a_sb is split into AG groups of DCH//AG chunks
    AGC = DCH // AG

    nc = bacc.Bacc("TRN2", target_bir_lowering=False, debug=False)

    # expansion matrices: m_np[e, kc*P + k] = 1 iff e == kc*16 + k//8
    m_np = np.zeros((E, KCH * P), np.float16)
    for kc in range(KCH):
        k = np.arange(P)
        m_np[kc * 16 + k // R, kc * P + k] = 1.0

    x_in = nc.dram_tensor("x", [tc_tokens, D], f32, kind="ExternalInput")
    a_in = nc.dram_tensor("a_t", [D, ER], f32r, kind="ExternalInput")     # A_w^T
    b_in = nc.dram_tensor("b_t", [ER, D], f32r, kind="ExternalInput")     # B_w^T * SCALING
    w_in = nc.dram_tensor("w_comb", [D, E], f32, kind="ExternalInput")    # Wr1 @ Wr2
    out_t = nc.dram_tensor("out", [tc_tokens, D], f32, kind="ExternalOutput")
    stats_t = nc.dram_tensor("stats", [1, 2 * E], f32, kind="ExternalOutput")
    m_dram = nc.inline_tensor(m_np, "m_expand")

    with tile.TileContext(nc) as tctx, ExitStack() as ctx:
        const_pool = ctx.enter_context(tctx.tile_pool(name="const", bufs=1))
        w_pool = ctx.enter_context(tctx.tile_pool(name="wts", bufs=1))
        xnat_pool = ctx.enter_context(tctx.tile_pool(name="xnat", bufs=20))
        xt32_pool = ctx.enter_context(tctx.tile_pool(name="xt32", bufs=17))
        xtr_pool = ctx.enter_context(tctx.tile_pool(name="xtr", bufs=10))
        zg_pool = ctx.enter_context(tctx.tile_pool(name="zg", bufs=6))
        ge_pool = ctx.enter_context(tctx.tile_pool(name="ge", bufs=6))
        gt_pool = ctx.enter_context(tctx.tile_pool(name="gt", bufs=2))
        sm_pool = ctx.enter_context(tctx.tile_pool(name="sm", bufs=3))
        outs_pool = ctx.enter_context(tctx.tile_pool(name="outs", bufs=4))

        ptlg_psum = ctx.enter_context(tctx.tile_pool(name="ptlg", bufs=2, space="PSUM"))
        zout_psum = ctx.enter_context(tctx.tile_pool(name="zop", bufs=4, space="PSUM"))
        outp_psum = ctx.enter_context(tctx.tile_pool(name="outp", bufs=2, space="PSUM"))

        # ---- constants / weights ----
        identity = const_pool.tile([P, P], f32)
        masks.make_identity(nc, identity[:])
        ones = const_pool.tile([P, 1], f32)
        nc.vector.memset(ones[:], 1.0)
        m16 = const_pool.tile([E, KCH * P], f16)
        nc.sync.dma_start(m16[:], m_dram[:])
        stats_acc = const_pool.tile([1, 2 * E], f32)
        nc.vector.memset(stats_acc[:], 0.0)

        # Weight tiles; DMAs are interleaved with the first x loads below so
        # the transpose/z pipeline starts early (the DMA engines serialize).
        a_sb = []
        for g in range(AG):
            ag = w_pool.tile([P, AGC * ER], f32r, tag=f"a{g}")
            a_sb.append(ag)
        w_sb = w_pool.tile([P, DCH * E], f32)
        b_sb = w_pool.tile([P, KCH * D], f32r)

        def load_xnat_quarter(xnat, t0, q):
            for s in range(NSUB):
                xq = xnat_pool.tile([P, 512], f32, tag="xnat", name=f"xq{t0}_{q}_{s}")
                nc.sync.dma_start(
                    xq[:],
                    x_in[t0 + s * P : t0 + (s + 1) * P, q * 512 : (q + 1) * 512],
                )
                xnat[s].append(xq)

        def emit_out(zg, t0o):
            # out = zg^T.T @ B^T per subtile x o-chunk (f32r)
            for s in range(NSUB):
                for oc in range(OCH):
                    op = outp_psum.tile([P, 512], f32, tag="outp", name=f"op{t0o}_{s}_{oc}")
                    for kc in range(KCH):
                        nc.tensor.matmul(
                            op[:],
                            lhsT=zg[kc][:, s * P : (s + 1) * P],
                            rhs=b_sb[:, kc * D + oc * 512 : kc * D + (oc + 1) * 512],
                            start=(kc == 0),
                            stop=(kc == KCH - 1),
                        )
                    o_sb = outs_pool.tile([P, 512], f32, tag="outs", name=f"os{t0o}_{s}_{oc}")
                    if oc % 2 == 0:
                        nc.scalar.activation(o_sb[:], op[:], FX.Copy)
                    else:
                        nc.vector.tensor_copy(o_sb[:], op[:])
                    nc.sync.dma_start(
                        out_t[t0o + s * P : t0o + (s + 1) * P, oc * 512 : (oc + 1) * 512],
                        o_sb[:],
                    )

        def load_a_group(g):
            # a_sb[g][p, jj*ER + k] = A^T[(g*AGC+jj)*128+p, k]
            nc.sync.dma_start(
                a_sb[g][:],
                a_in[g * AGC * P : (g + 1) * AGC * P, :].rearrange(
                    "(j p) k -> p j k", p=P
                ),
            )

        for mt in range(MT):
            t0 = mt * TT
            # ---- load x natural as 4x4 quarter tiles [128, 512] ----
            xnat = [[] for _ in range(NSUB)]
            if mt == 0:
                # interleave first x tile loads with weight loads
                load_xnat_quarter(xnat, t0, 0)
                load_a_group(0)
                nc.sync.dma_start(w_sb[:], w_in.rearrange("(j p) e -> p j e", p=P))
                load_xnat_quarter(xnat, t0, 1)
                load_a_group(1)
                load_xnat_quarter(xnat, t0, 2)
                load_a_group(2)
                load_xnat_quarter(xnat, t0, 3)
                load_a_group(3)
                # B^T k-chunk-major: b_sb[p, c*D + o] = B^T[c*128+p, o]
                nc.sync.dma_start(b_sb[:], b_in.rearrange("(c p) o -> p c o", p=P))
            else:
                for q in range(4):
                    load_xnat_quarter(xnat, t0, q)

            # ---- transpose + dual evict + z matmuls, j-interleaved ----
            xt32 = []
            zps = []
            for j in range(DCH):
                pt = ptlg_psum.tile([P, TT], f32, tag="ptlg")
                for s in range(NSUB):
                    nc.tensor.transpose(
                        pt[:, s * P : (s + 1) * P],
                        xnat[s][j // 4][:, (j % 4) * P : (j % 4 + 1) * P],
                        identity[:],
                    )
                xtrj = xtr_pool.tile([P, TT], f32r, tag="xtr")
                nc.vector.tensor_copy(xtrj[:], pt[:])
                for kc in range(KCH):
                    if j == 0:
                        zpt = zout_psum.tile([P, TT], f32, tag="zop")
                        zps.append(zpt)
                    nc.tensor.matmul(
                        zps[kc][:],
                        lhsT=a_sb[j // AGC][:, (j % AGC) * ER + kc * P : (j % AGC) * ER + (kc + 1) * P],
                        rhs=xtrj[:],
                        start=(j == 0),
                        stop=(j == DCH - 1),
                    )
                xt32j = xt32_pool.tile([P, TT], f32, tag="xt32")
                nc.scalar.activation(xt32j[:], pt[:], FX.Copy)
                xt32.append(xt32j)

            # ---- previous macro-tile's out matmuls (fills the gate tail) ----
            if mt > 0:
                emit_out(prev_zg, (mt - 1) * TT)

            # ---- router chains s0..s3 (PE, independent of softmax) ----
            lg = [None] * NSUB
            for s in range(NSUB):
                lg[s] = ptlg_psum.tile([P, E], f32, tag="ptlg", name=f"lg{s}_{mt}")
                for j in range(DCH):
                    nc.tensor.matmul(
                        lg[s][:],
                        lhsT=xt32[j][:, s * P : (s + 1) * P],
                        rhs=w_sb[:, j * E : (j + 1) * E],
                        start=(j == 0),
                        stop=(j == DCH - 1),
                    )

            # ---- per-s softmax + top16 (DVE/ACT only; top-k on exps) ----
            gates = []
            stat_srcs = []
            for s in range(NSUB):
                nmax = sm_pool.tile([P, 1], f32, tag="nmax")
                nc.vector.reduce_max(nmax[:], lg[s][:], axis=AX.X, negate=True)
                exps = sm_pool.tile([P, E], f32, tag="exps", bufs=5)
                sumx = sm_pool.tile([P, 1], f32, tag="sumx")
                nc.scalar.activation(
                    exps[:], lg[s][:], FX.Exp, bias=nmax[:, 0:1], scale=1.0,
                    accum_out=sumx[:, 0:1],
                )
                rec = sm_pool.tile([P, 1], f32, tag="rec", bufs=5)
                nc.vector.reciprocal(rec[:], sumx[:])
                # top-16 of exps (same selection as probs; softmax monotonic)
                m8a = sm_pool.tile([P, 8], f32, tag="m8a")
                res1 = sm_pool.tile([P, E], f32, tag="res1")
                nc.vector.max(m8a[:], exps[:])
                nc.vector.match_replace(res1[:], m8a[:], exps[:], 0.0)
                m8b = sm_pool.tile([P, 8], f32, tag="m8b")
                res2 = sm_pool.tile([P, E], f32, tag="res2", bufs=5)
                nc.vector.max(m8b[:], res1[:])
                nc.vector.match_replace(res2[:], m8b[:], res1[:], 0.0)
                gate_u = sm_pool.tile([P, E], f32, tag="gate_u", bufs=5)
                nc.vector.tensor_sub(gate_u[:], exps[:], res2[:])
                gate = sm_pool.tile([P, E], f32, tag="gate", bufs=5)
                nc.vector.tensor_scalar_mul(gate[:], gate_u[:], rec[:, 0:1])
                stat_src = sm_pool.tile([P, 2 * E], f32, tag="stat", bufs=5)
                nc.vector.tensor_scalar_mul(stat_src[:, :E], exps[:], rec[:, 0:1])
                nc.gpsimd.tensor_single_scalar(
                    stat_src[:, E:], gate_u[:], 0.0, ALU.is_gt
                )
                gates.append(gate)
                stat_srcs.append(stat_src)

            # ---- gate transposes (PE; wait on DVE) -> fp16 gt_sb ----
            gt_sb = gt_pool.tile([E, TT], f16, tag="gt")
            gtps = []
            for s in range(NSUB):
                gtp = ptlg_psum.tile([E, P], f32, tag="ptlg")
                nc.tensor.transpose(gtp[:], gates[s][:], identity[:])
                gtps.append(gtp)
            for s in range(NSUB):
                nc.vector.tensor_copy(gt_sb[:, s * P : (s + 1) * P], gtps[s][:])

            # ---- expand gate to k-rows (fp16 matmul) + gate z (f32r out) ----
            zg = []
            for kc in range(KCH):
                gep = ptlg_psum.tile([P, TT], f32, tag="ptlg")
                nc.tensor.matmul(
                    gep[:], lhsT=m16[:, kc * P : (kc + 1) * P], rhs=gt_sb[:]
                )
                ge_sb = ge_pool.tile([P, TT], f32, tag="ge")
                nc.scalar.activation(ge_sb[:], gep[:], FX.Copy)
                zg_sb = zg_pool.tile([P, TT], f32r, tag="zg")
                nc.vector.tensor_mul(zg_sb[:], zps[kc][:], ge_sb[:])
                zps[kc] = None
                zg.append(zg_sb)

            # ---- stats col-sums (PE; ready early, fills gate-chain latency) ----
            for s in range(NSUB):
                st = ptlg_psum.tile([1, 2 * E], f32, tag="ptlg")
                nc.tensor.matmul(st[:], lhsT=ones[:], rhs=stat_srcs[s][:])
                nc.vector.tensor_add(stats_acc[:], stats_acc[:], st[:])

            prev_zg = zg

        emit_out(prev_zg, (MT - 1) * TT)
        nc.sync.dma_start(stats_t[:], stats_acc[:])

    nc.compile()
    return nc


def _get_nc(tc_tokens=TC):
    if tc_tokens not in _BUILT:
        _BUILT[tc_tokens] = _build_nc(tc_tokens)
    return _BUILT[tc_tokens]


def _host_prep(A_w, B_w, Wr1, Wr2):
    a_t = np.ascontiguousarray(np.asarray(A_w, np.float32).T)            # (D, ER)
    b_t = np.ascontiguousarray(np.asarray(B_w, np.float32).T) * np.float32(SCALING)  # (ER, D)
    w_comb = (np.asarray(Wr1, np.float64) @ np.asarray(Wr2, np.float64)).astype(np.float32)  # (D, E)
    return a_t, b_t, w_comb


def kernel(x, A_w, B_w, Wr1, Wr2):
    from concourse.bass_utils import run_bass_kernel_spmd

    x = np.asarray(x, np.float32)
    a_t, b_t, w_comb = _host_prep(A_w, B_w, Wr1, Wr2)
    xf = np.ascontiguousarray(x.reshape(T, D))

    nc = _get_nc(TC)
    in_maps = []
    for c in range(NCORES):
        in_maps.append({
            "x": np.ascontiguousarray(xf[c * TC : (c + 1) * TC]),
            "a_t": a_t,
            "b_t": b_t,
            "w_comb": w_comb,
        })
    res = run_bass_kernel_spmd(nc, in_maps, core_ids=list(range(NCORES)))
    outs = res.results

    out = np.concatenate([r["out"] for r in outs], axis=0).reshape(B, S, D)
    stats = np.sum([r["stats"][0] for r in outs], axis=0)
    p_sum = stats[:E]
    f_sum = stats[E:]
    aux = np.float32(E * np.sum((f_sum / T) * (p_sum / T)))
    return out, aux
